# revision 26
# baseline (speedup 1.0000x reference)
"""Trainium2 Bass kernel for nn_DRGAN (gnn_message_passing).

3-block residual-gated GraphSAGE GNN, N=50000 nodes, E=800000 edges, H=256.

Strategy (8 NeuronCores, SPMD single NEFF):
 - Nodes sharded by destination: core c owns nodes [c*6250, (c+1)*6250).
 - Activations live feature-major ([feat, node]) in SBUF as bf16; all matmul
   math accumulates fp32 in PSUM; BN/bias folded into ACT scale/bias.
 - Per SAGE layer: the full feature table is replicated per-core in DRAM
   (bf16, node-major) via two AllGathers (lo/hi node halves -> int16 gather
   indices fit, and the two collectives pipeline against compute).
 - segment-mean: edges sorted by (dst block of 128, src chunk); per block,
   dma_gather pulls source rows into SBUF ([128 edges, 256] per subtile);
   a one-hot selection matrix S (built on DVE from precomputed seg ids via
   is_equal against an iota row) maps edges->dsts with a PE matmul
   accumulating into PSUM; inv-degree scaling on ACT gives the mean.
"""

import os
import numpy as np

import concourse.bacc as bacc
import concourse.bass as bass
import concourse.mybir as mybir
import concourse.tile as tile
from concourse.bass_utils import run_bass_kernel_spmd
from concourse.masks import make_identity

# ---------------- problem constants (hardcoded per contract) ----------------
N, E, D, H, C, NBLK, NGATE = 50000, 800000, 128, 256, 40, 3, 2
EPS = 1e-5
NCORES = 8
NPC = N // NCORES            # 6250 nodes per core
BW = 128                     # dst block width
NB = (NPC + BW - 1) // BW    # 49 dst blocks per core (48 full + 106)
LOB = 25                     # blocks 0..24 -> "lo" table half
LO_ROWS = LOB * BW           # 3200 rows/core in lo half
HI_ROWS = NPC - LO_ROWS      # 3050 rows/core in hi half
NSEG = 6                     # SAGE layers total (3 blocks x 2)

BF16 = mybir.dt.bfloat16
FP32 = mybir.dt.float32
I16 = mybir.dt.int16
AF = mybir.ActivationFunctionType
ALU = mybir.AluOpType

_bf16 = np.dtype("bfloat16") if hasattr(np, "bfloat16") else None
import ml_dtypes  # noqa: E402
NP_BF16 = ml_dtypes.bfloat16


def _blkw(b):
    return BW if b < NB - 1 else NPC - (NB - 1) * BW


# ============================================================================
# host-side preprocessing
# ============================================================================

def _preprocess(x, edge_index):
    """Build per-core gather indices / seg ids / inv counts + nsub tables."""
    src = np.asarray(edge_index[0], dtype=np.int64)
    dst = np.asarray(edge_index[1], dtype=np.int64)

    deg = np.bincount(dst, minlength=N).astype(np.float32)
    inv_cnt_full = 1.0 / np.maximum(deg, 1.0)

    # table row mapping (lo/hi split by within-slice row)
    s_rank = src // NPC
    s_row = src % NPC
    is_hi = s_row >= LO_ROWS
    tbl_row = np.where(is_hi, s_rank * HI_ROWS + (s_row - LO_ROWS),
                       s_rank * LO_ROWS + s_row).astype(np.int64)

    core = dst // NPC
    dloc = dst % NPC
    blk = dloc // BW
    seg = dloc % BW

    # group edges by (core, block, chunk); sort by tbl_row for DRAM locality
    order = np.lexsort((tbl_row, is_hi.astype(np.int64), blk, core))
    core_s, blk_s = core[order], blk[order]
    hi_s, row_s, seg_s = is_hi[order], tbl_row[order], seg[order]

    # counts[core, blk, chunk]
    counts = np.zeros((NCORES, NB, 2), dtype=np.int64)
    np.add.at(counts, (core_s, blk_s, hi_s.astype(np.int64)), 1)
    nsub = np.ceil(counts.max(axis=0) / 128.0).astype(np.int64)  # [NB, 2]
    nsub = np.maximum(nsub, 1)

    nsub_tot = int(nsub.sum())
    w16 = nsub_tot * 8

    idx_all = np.zeros((NCORES, 128, w16), dtype=np.int16)
    seg_all = np.full((NCORES, 128, nsub_tot), -1.0, dtype=NP_BF16)
    icnt = np.ones((NCORES, 128, NB), dtype=np.float32)

    # start offset of each (core,blk,chunk) run inside the sorted arrays
    starts = np.zeros((NCORES, NB, 2), dtype=np.int64)
    flat_key = (core_s * NB + blk_s) * 2 + hi_s.astype(np.int64)
    uniq, first = np.unique(flat_key, return_index=True)
    run_start = dict(zip(uniq.tolist(), first.tolist()))

    scol = np.zeros((NB, 2), dtype=np.int64)   # subtile col offsets (global)
    ccol = np.zeros((NB, 2), dtype=np.int64)   # idx col offsets
    acc = 0
    for b in range(NB):
        for h in range(2):
            scol[b, h] = acc
            ccol[b, h] = acc * 8
            acc += nsub[b, h]

    for c in range(NCORES):
        for b in range(NB):
            for h in range(2):
                cnt = int(counts[c, b, h])
                ns = int(nsub[b, h])
                cap = ns * 128
                rows = np.zeros(cap, dtype=np.int16)
                segs = np.full(cap, -1.0, dtype=np.float32)
                if cnt:
                    st = run_start[(c * NB + b) * 2 + h]
                    rows[:cnt] = row_s[st:st + cnt].astype(np.int16)
                    segs[:cnt] = seg_s[st:st + cnt].astype(np.float32)
                if os.environ.get("K_SEQIDX", "0") == "1":
                    tblrows = NCORES * (LO_ROWS if h == 0 else HI_ROWS)
                    rows = ((b * 1000 + np.arange(cap)) % tblrows).astype(np.int16)
                # idx layout: edge i -> [i % 16, ccol + i // 16], replicated x8
                a2 = rows.reshape(ns * 8, 16).T            # [16, ns*8]
                idx_all[c, :, ccol[b, h]:ccol[b, h] + ns * 8] = np.tile(a2, (8, 1))
                # seg layout: edge i -> [i % 128, scol + i // 128]
                s2 = segs.reshape(ns, 128).T               # [128, ns]
                seg_all[c, :, scol[b, h]:scol[b, h] + ns] = s2.astype(NP_BF16)
        ic = inv_cnt_full[c * NPC:(c + 1) * NPC]
        icp = np.ones(NB * BW, dtype=np.float32)
        icp[:NPC] = ic
        icnt[c] = icp.reshape(NB, BW).T

    xT = np.ascontiguousarray(x.reshape(NCORES, NPC, D).transpose(0, 2, 1)).astype(np.float32)

    return dict(idx_all=idx_all, seg_all=seg_all, icnt=icnt, xT=xT,
                nsub=nsub, scol=scol, ccol=ccol, w16=w16, nsub_tot=nsub_tot)


def _pack_weights(inp):
    """Pack all matmul weights (as lhsT [in, out]) into one [128, WCOL] bf16."""
    cols = []
    ofs = {}

    def add(name, w):
        w = np.asarray(w, dtype=np.float32)
        K, M = w.shape
        assert K % 128 == 0
        ofs[name] = sum(c.shape[1] for c in cols)
        for kc in range(K // 128):
            cols.append(w[kc * 128:(kc + 1) * 128, :])

    add("Wp", inp["Wp"])                       # [128, 256]
    for i in range(NBLK):
        add(f"W1l{i}", inp["W1l"][i])
        add(f"W1r{i}", inp["W1r"][i])
        add(f"W2l{i}", inp["W2l"][i])
        add(f"W2r{i}", inp["W2r"][i])
    for g in range(NGATE):
        add(f"gW{g}", inp["gate_W"][g])
    add("Wc1", inp["Wc1"])                     # [256, 128]
    add("Wc2", inp["Wc2"])                     # [128, 40]
    wcat = np.concatenate(cols, axis=1).astype(np.float32)
    return wcat, ofs


def _pack_params(inp):
    """Per-feature params as [128, PCOL] fp32 columns (feat-major halves)."""
    cols = []
    ofs = {}

    def add(name, v, parts=2):
        v = np.asarray(v, dtype=np.float32).reshape(-1)
        ofs[name] = len(cols) * 0 + sum(c.shape[1] for c in cols)
        if parts == 2:
            assert v.size == 256
            cols.append(np.stack([v[:128], v[128:]], axis=1))
        else:
            pad = np.zeros(128, np.float32)
            pad[:v.size] = v
            cols.append(pad[:, None])

    add("bp", inp["bp"])
    for i in range(NBLK):
        for ln, (g, b, m, v, bias) in {
            1: (inp["bn1_g"][i], inp["bn1_b"][i], inp["bn1_m"][i], inp["bn1_v"][i], inp["b1l"][i]),
            2: (inp["bn2_g"][i], inp["bn2_b"][i], inp["bn2_m"][i], inp["bn2_v"][i], inp["b2l"][i]),
        }.items():
            k = g / np.sqrt(v + EPS)
            cc = b + (bias - m) * k
            add(f"k{ln}_{i}", k)
            add(f"c{ln}_{i}", cc)
    for g in range(NGATE):
        add(f"gb{g}", inp["gate_b"][g])
    add("bc1", inp["bc1"], parts=1)
    add("bc2", inp["bc2"], parts=1)
    pcat = np.concatenate(cols, axis=1).astype(np.float32)
    return pcat, ofs


# ============================================================================
# device program
# ============================================================================

def _bcast(ap, reps, axis_pos):
    """Insert a broadcast (step 0) axis of length `reps` at axis_pos."""
    lst = [list(p) for p in ap.ap]
    lst.insert(axis_pos, [0, reps])
    return bass.AP(ap.tensor, ap.offset, lst)


def _build(meta, w16, nsub_tot, nsub, scol, ccol, wofs, pofs, wcol, pcol):
    nc = bacc.Bacc("TRN2", target_bir_lowering=False, debug=False,
                   enable_asserts=False, num_devices=NCORES,
                   num_swdge_queues=int(os.environ.get("K_SWQ", "1")))

    # ---- I/O
    xT_d = nc.dram_tensor("xT", [D, NPC], FP32, kind="ExternalInput")
    idx_d = nc.dram_tensor("idx_all", [128, w16], I16, kind="ExternalInput")
    seg_d = nc.dram_tensor("seg_all", [128, nsub_tot], BF16, kind="ExternalInput")
    icnt_d = nc.dram_tensor("icnt", [128, NB], FP32, kind="ExternalInput")
    w_d = nc.dram_tensor("wcat", [128, wcol], FP32, kind="ExternalInput")
    p_d = nc.dram_tensor("pcat", [128, pcol], FP32, kind="ExternalInput")
    out_d = nc.dram_tensor("out_own", [NPC, C], FP32, kind="ExternalOutput")
    stage = os.environ.get("K_STAGE", "full")
    dbg_d = (nc.dram_tensor("dbg", [2 * 128, NPC], FP32, kind="ExternalOutput")
             if stage != "full" else None)

    # ---- DRAM internals: ping-pong gather tables + AG staging
    # Tables hold bf16x2 split rows: [hi(256) | lo(256)] bf16, 1KB per row.
    H2 = 2 * H
    tbl_lo = [nc.dram_tensor(f"tbl_lo{i}", [NCORES * LO_ROWS, H2], BF16,
                             kind="Internal", addr_space="Shared") for i in range(2)]
    tbl_hi = [nc.dram_tensor(f"tbl_hi{i}", [NCORES * HI_ROWS, H2], BF16,
                             kind="Internal", addr_space="Shared") for i in range(2)]
    agin_lo = nc.dram_tensor("agin_lo", [LO_ROWS, H2], BF16, kind="Internal")
    agin_hi = nc.dram_tensor("agin_hi", [HI_ROWS, H2], BF16, kind="Internal")

    NCH = [(j * 512, min(512, NPC - j * 512)) for j in range((NPC + 511) // 512)]
    RG = [list(range(NCORES))]

    with tile.TileContext(nc) as tc:
        with (
            tc.tile_pool(name="res", bufs=1) as res,
            tc.tile_pool(name="gp", bufs=2) as gp,
            tc.tile_pool(name="ixp", bufs=3) as ixp,
            tc.tile_pool(name="sp", bufs=2) as sp,
            tc.tile_pool(name="nmp", bufs=2) as nmp,
            tc.tile_pool(name="smp", bufs=2) as smp,
            tc.tile_pool(name="mtp", bufs=2) as mtp,
            tc.tile_pool(name="h2p", bufs=2) as h2p,
            tc.tile_pool(name="ldp", bufs=2) as ldp,
            tc.tile_pool(name="gtp", bufs=2) as gtp,
            tc.tile_pool(name="psA", bufs=2, space="PSUM") as psA,
            tc.tile_pool(name="psT", bufs=3, space="PSUM") as psT,
            tc.tile_pool(name="psM", bufs=3, space="PSUM") as psM,
        ):
            # ---------- resident tiles
            wsb = res.tile([128, wcol], FP32)
            nc.sync.dma_start(wsb[:], w_d[:])
            psb = res.tile([128, pcol], FP32)
            nc.sync.dma_start(psb[:], p_d[:])
            seg_sb = res.tile([128, nsub_tot], BF16)
            nc.sync.dma_start(seg_sb[:], seg_d[:])
            icnt_sb = res.tile([128, NB], FP32)
            nc.sync.dma_start(icnt_sb[:], icnt_d[:])

            iota_sb = res.tile([128, 128], BF16)
            nc.gpsimd.iota(iota_sb[:], pattern=[[1, 128]], base=0,
                           channel_multiplier=0, allow_small_or_imprecise_dtypes=True)
            idn_f32 = res.tile([128, 128], FP32)
            make_identity(nc, idn_f32[:])

            # h storage, feature-major halves [128 feats, NPC nodes]
            hcur = [res.tile([128, NPC], FP32, name=f"hcur{h}", tag=f"hcur{h}") for h in range(2)]
            t1 = [res.tile([128, NPC], FP32, name=f"t1_{h}", tag=f"t1_{h}") for h in range(2)]

            def wtile(name, kh, mh, mwidth=128, M=256):
                o = wofs[name] + kh * M + mh * mwidth
                return wsb[:, o:o + mwidth]

            def pcolap(name, half, rows=128):
                return psb[0:rows, pofs[name] + half:pofs[name] + half + 1]

            def publish_block(src_block_aps, b, pub):
                """src_block_aps: [2] feat-major [128, w] APs for block b.
                Emits bf16x2 rows: nm[:, 0:H] = hi(v), nm[:, H:2H] = lo(v)."""
                w = _blkw(b)
                nm = nmp.tile([128, H2], BF16, name="nm", tag="nmq")
                for hf in range(2):
                    pt = psT.tile([128, 128], FP32, name="pt", tag="pt", space="PSUM")
                    nc.tensor.transpose(pt[0:w, :], src_block_aps[hf], idn_f32[:])
                    hi_ap = nm[0:w, hf * 128:(hf + 1) * 128]
                    nc.vector.tensor_copy(out=hi_ap, in_=pt[0:w, :])
                    nc.vector.tensor_tensor(
                        out=nm[0:w, H + hf * 128:H + (hf + 1) * 128],
                        in0=pt[0:w, :], in1=hi_ap, op=ALU.subtract)
                if b < LOB:
                    nc.sync.dma_start(agin_lo[b * BW:b * BW + w, :], nm[0:w, :])
                else:
                    r0 = (b - LOB) * BW
                    nc.sync.dma_start(agin_hi[r0:r0 + w, :], nm[0:w, :])

            def ag_fire(pub, half):
                tl, th = tbl_lo[pub % 2], tbl_hi[pub % 2]
                if os.environ.get("K_NOAG", "0") == "1":
                    # timing experiment: local copy instead of AllGather
                    if half == 0:
                        nc.sync.dma_start(tl[0:LO_ROWS, :], agin_lo[:])
                    else:
                        nc.sync.dma_start(th[0:HI_ROWS, :], agin_hi[:])
                    return
                if half == 0:
                    nc.gpsimd.collective_compute("AllGather", ALU.bypass,
                                                 replica_groups=RG,
                                                 ins=[agin_lo[:]], outs=[tl[:]])
                else:
                    nc.gpsimd.collective_compute("AllGather", ALU.bypass,
                                                 replica_groups=RG,
                                                 ins=[agin_hi[:]], outs=[th[:]])

            def block_mean(b, pub):
                """Gather + one-hot matmul + inv-cnt scale + transpose.
                Returns [2] feat-major mean tiles [128, 128]."""
                tl, th = tbl_lo[pub % 2], tbl_hi[pub % 2]
                abl = os.environ.get("K_ABL", "")
                nlo, nhi = int(nsub[b, 0]), int(nsub[b, 1])
                ns = nlo + nhi
                g = gp.tile([128, ns, H2], BF16, name="g", tag="g")
                c0 = int(ccol[b, 0])
                idxb = ixp.tile([128, ns * 8], I16, name="idxb", tag="idxb")
                nc.sync.dma_start(idxb[:], idx_d[:, c0:c0 + ns * 8])
                if abl == "nogather":
                    nc.sync.dma_start(
                        g[:, 0:nlo, :],
                        tl[0:nlo * 128, :].rearrange("(t p) f -> p t f", p=128))
                    nc.sync.dma_start(
                        g[:, nlo:ns, :],
                        th[0:nhi * 128, :].rearrange("(t p) f -> p t f", p=128))
                elif abl == "onegather":
                    nc.gpsimd.dma_gather(
                        g[:, 0:ns, :], tl[:], idxb[:, 0:ns * 8],
                        ns * 128, ns * 128, H2, single_packet=False)
                else:
                    nq = int(os.environ.get("K_SWQ", "1"))
                    nc.gpsimd.dma_gather(
                        g[:, 0:nlo, :], tl[:], idxb[:, 0:nlo * 8],
                        nlo * 128, nlo * 128, H2, single_packet=(nlo * 128 <= 1024),
                        queue_num=(2 * b) % nq)
                    nc.gpsimd.dma_gather(
                        g[:, nlo:ns, :], th[:], idxb[:, nlo * 8:ns * 8],
                        nhi * 128, nhi * 128, H2, single_packet=(nhi * 128 <= 1024),
                        queue_num=(2 * b + 1) % nq)
                s = sp.tile([128, ns * 128], BF16, name="s", tag="s")
                s3 = s[:].rearrange("p (t f) -> p t f", t=ns)
                segb = _bcast(seg_sb[:, int(scol[b, 0]):int(scol[b, 0]) + ns], 128, 2)
                iob = _bcast(iota_sb[:], ns, 1)
                if abl != "nosbuild":
                    nc.vector.tensor_tensor(out=s3, in0=segb, in1=iob, op=ALU.is_equal)
                ps = psA.tile([128, H], FP32, name="agg", tag="agg", space="PSUM")
                nsub_eff = 1 if abl == "noagg" else ns
                for t in range(nsub_eff):
                    nc.tensor.matmul(ps[:], lhsT=s[:, t * 128:(t + 1) * 128],
                                     rhs=g[:, t, 0:H], start=(t == 0), stop=False)
                    nc.tensor.matmul(ps[:], lhsT=s[:, t * 128:(t + 1) * 128],
                                     rhs=g[:, t, H:H2], start=False,
                                     stop=(t == nsub_eff - 1))
                mn = nmp.tile([128, H], FP32, name="mn", tag="nm")
                nc.scalar.mul(mn[:], ps[:], icnt_sb[:, b:b + 1])
                return mn

            def mean_transpose(mn):
                mts = []
                for hf in range(2):
                    pt = psT.tile([128, 128], FP32, name="ptm", tag="pt", space="PSUM")
                    nc.tensor.transpose(pt[:], mn[:, hf * 128:(hf + 1) * 128], idn_f32[:])
                    mt = mtp.tile([128, 128], FP32, name="mt", tag=f"mt{hf}")
                    nc.vector.tensor_copy(out=mt[:], in_=pt[:])
                    mts.append(mt)
                return mts

            def sage_layer(pub_in, pub_out, x_src, dst, wl, wr, kname, cname,
                           combine_i=None):
                """One SAGE conv, fused per dst block, software-pipelined:
                block b+1's gather/agg issues before block b's finish so the
                PE queue never stalls on the mean's ACT/DVE hops.
                dst: t1 array (sage1) or None (sage2: combine into hcur).
                pub_out: publish index or None."""
                mns = {}
                mns[0] = block_mean(0, pub_in)
                for bb in range(NB):
                    if bb + 1 < NB:
                        mns[bb + 1] = block_mean(bb + 1, pub_in)
                    b = bb
                    w = _blkw(b)
                    bsl = slice(b * BW, b * BW + w)
                    mts = mean_transpose(mns.pop(b))
                    outs = []
                    nodense = os.environ.get("K_ABL", "") == "nodense"
                    for mh in range(2):
                        ps = psM.tile([128, 512], FP32, name="mm", tag="mm", space="PSUM")
                        nc.tensor.matmul(ps[:, 0:w], lhsT=wtile(wl, 0, mh),
                                         rhs=mts[0][:, 0:w], start=True,
                                         stop=nodense)
                        if not nodense:
                            nc.tensor.matmul(ps[:, 0:w], lhsT=wtile(wl, 1, mh),
                                             rhs=mts[1][:, 0:w], start=False, stop=False)
                            nc.tensor.matmul(ps[:, 0:w], lhsT=wtile(wr, 0, mh),
                                             rhs=x_src[0][:, bsl], start=False, stop=False)
                            nc.tensor.matmul(ps[:, 0:w], lhsT=wtile(wr, 1, mh),
                                             rhs=x_src[1][:, bsl], start=False, stop=True)
                        if dst is not None:
                            nc.scalar.activation(dst[mh][:, bsl], ps[:, 0:w],
                                                 AF.Relu, bias=pcolap(cname, mh),
                                                 scale=pcolap(kname, mh))
                        else:
                            h2 = h2p.tile([128, 128], FP32, name="h2", tag=f"h2_{mh}")
                            nc.scalar.activation(h2[:, 0:w], ps[:, 0:w],
                                                 AF.Relu, bias=pcolap(cname, mh),
                                                 scale=pcolap(kname, mh))
                            outs.append(h2)
                    if dst is None:
                        # combine into hcur
                        if combine_i == 0:
                            for mh in range(2):
                                nc.vector.tensor_tensor(
                                    out=hcur[mh][:, bsl], in0=hcur[mh][:, bsl],
                                    in1=outs[mh][:, 0:w], op=ALU.add)
                        else:
                            gts = []
                            for mh in range(2):
                                ps = psM.tile([128, 512], FP32, name="gmm", tag="mm",
                                              space="PSUM")
                                nc.tensor.matmul(ps[:, 0:w],
                                                 lhsT=wtile(f"gW{combine_i - 1}", 0, mh),
                                                 rhs=hcur[0][:, bsl], start=True, stop=False)
                                nc.tensor.matmul(ps[:, 0:w],
                                                 lhsT=wtile(f"gW{combine_i - 1}", 1, mh),
                                                 rhs=hcur[1][:, bsl], start=False, stop=True)
                                gt = gtp.tile([128, 128], FP32, name="gt", tag="gt")
                                nc.scalar.activation(gt[:, 0:w], ps[:, 0:w], AF.Sigmoid,
                                                     bias=pcolap(f"gb{combine_i - 1}", mh))
                                gts.append(gt)
                            for mh in range(2):
                                u = gtp.tile([128, 128], FP32, name="u", tag="u")
                                nc.vector.tensor_tensor(out=u[:, 0:w], in0=gts[mh][:, 0:w],
                                                        in1=outs[mh][:, 0:w], op=ALU.mult)
                                nc.vector.tensor_tensor(out=u[:, 0:w],
                                                        in0=outs[mh][:, 0:w],
                                                        in1=u[:, 0:w], op=ALU.subtract)
                                nc.vector.tensor_tensor(out=hcur[mh][:, bsl],
                                                        in0=hcur[mh][:, bsl],
                                                        in1=u[:, 0:w], op=ALU.add)
                    if pub_out is not None:
                        src = dst if dst is not None else hcur
                        publish_block([src[hf][:, bsl] for hf in range(2)], b, pub_out)
                        if b == LOB - 1:
                            ag_fire(pub_out, 0)
                        elif b == NB - 1:
                            ag_fire(pub_out, 1)

            def dump(src):
                for hf in range(2):
                    nc.sync.dma_start(dbg_d[hf * 128:(hf + 1) * 128, :], src[hf][:])

            # ================= program =================
            # input projection: hcur = x @ Wp + bp   (K = D = 128)
            for (j0, jw) in NCH:
                xc = ldp.tile([128, 512], FP32, name="xc", tag="xc")
                nc.sync.dma_start(xc[:, 0:jw], xT_d[:, j0:j0 + jw])
                for mh in range(2):
                    ps = psM.tile([128, 512], FP32, name="pj", tag="mm", space="PSUM")
                    nc.tensor.matmul(ps[:, 0:jw], lhsT=wtile("Wp", 0, mh),
                                     rhs=xc[:, 0:jw], start=True, stop=True)
                    nc.scalar.activation(hcur[mh][:, j0:j0 + jw], ps[:, 0:jw],
                                         AF.Identity, bias=pcolap("bp", mh))
            if stage == "proj":
                dump(hcur)
            else:
                for b in range(NB):
                    publish_block([hcur[hf][:, b * BW:b * BW + _blkw(b)]
                                   for hf in range(2)], b, 0)
                ag_fire(0, 0)
                ag_fire(0, 1)

            if stage == "sage1":
                sage_layer(0, None, hcur, t1, "W1l0", "W1r0", "k1_0", "c1_0")
                dump(t1)
            elif stage == "full":
                pub = 0
                for i in range(NBLK):
                    sage_layer(pub, pub + 1, hcur, t1,
                               f"W1l{i}", f"W1r{i}", f"k1_{i}", f"c1_{i}")
                    pub += 1
                    last = (i == NBLK - 1)
                    sage_layer(pub, None if last else pub + 1, t1, None,
                               f"W2l{i}", f"W2r{i}", f"k2_{i}", f"c2_{i}",
                               combine_i=i)
                    if not last:
                        pub += 1

                # head: out = relu(h @ Wc1 + bc1) @ Wc2 + bc2
                for (j0, jw) in NCH:
                    ps = psM.tile([128, 512], FP32, name="h1p", tag="mm", space="PSUM")
                    for kh in range(2):
                        nc.tensor.matmul(ps[:, 0:jw], lhsT=wtile("Wc1", kh, 0, M=128),
                                         rhs=hcur[kh][:, j0:j0 + jw],
                                         start=(kh == 0), stop=(kh == 1))
                    z = ldp.tile([128, 512], FP32, name="z", tag="xc")
                    nc.scalar.activation(z[:, 0:jw], ps[:, 0:jw], AF.Relu,
                                         bias=pcolap("bc1", 0))
                    o2 = gtp.tile([40, 512], FP32, name="o2", tag="o2")
                    ps2 = psM.tile([40, 512], FP32, name="h2p", tag="mm", space="PSUM")
                    nc.tensor.matmul(ps2[:, 0:jw], lhsT=wtile("Wc2", 0, 0, mwidth=C, M=C),
                                     rhs=z[:, 0:jw], start=True, stop=True)
                    nc.scalar.activation(o2[:, 0:jw], ps2[:, 0:jw],
                                         AF.Identity, bias=pcolap("bc2", 0, rows=40))
                    for sb_ in range(0, jw, 128):
                        b = (j0 + sb_) // BW
                        w = min(128, jw - sb_)
                        pt = psT.tile([128, 40], FP32, name="pto", tag="pt", space="PSUM")
                        nc.tensor.matmul(pt[0:w, :], lhsT=o2[:, sb_:sb_ + w],
                                         rhs=idn_f32[0:40, 0:40], is_transpose=True,
                                         start=True, stop=True)
                        ob = nmp.tile([128, C], FP32, name="ob", tag="ob")
                        nc.vector.tensor_copy(out=ob[0:w, :], in_=pt[0:w, :])
                        nc.sync.dma_start(out_d[j0 + sb_:j0 + sb_ + w, :], ob[0:w, :])

    nc.compile()
    return nc


# ============================================================================
# entry point
# ============================================================================

_CACHE = {}


def _get_program(prep, wcol, pcol, wofs, pofs):
    key = (prep["w16"], prep["nsub_tot"], wcol, pcol, os.environ.get("K_STAGE", "full"),
           os.environ.get("K_NOAG", "0"), os.environ.get("K_ABL", ""),
           os.environ.get("K_SWQ", "1"))
    if key not in _CACHE:
        _CACHE[key] = _build(None, prep["w16"], prep["nsub_tot"], prep["nsub"],
                             prep["scol"], prep["ccol"], wofs, pofs, wcol, pcol)
    return _CACHE[key]


def kernel(**inputs):
    x = np.asarray(inputs["x"], dtype=np.float32)
    prep = _preprocess(x, np.asarray(inputs["edge_index"]))
    wcat, wofs = _pack_weights(inputs)
    pcat, pofs = _pack_params(inputs)

    nc = _get_program(prep, wcat.shape[1], pcat.shape[1], wofs, pofs)

    in_maps = []
    for c in range(NCORES):
        in_maps.append({
            "xT": prep["xT"][c],
            "idx_all": prep["idx_all"][c],
            "seg_all": prep["seg_all"][c],
            "icnt": prep["icnt"][c],
            "wcat": wcat,
            "pcat": pcat,
        })
    res = run_bass_kernel_spmd(nc, in_maps, core_ids=list(range(NCORES)))
    out = np.concatenate([res.results[c]["out_own"] for c in range(NCORES)], axis=0)
    return out.astype(np.float32)


def time_kernel(reps=5, **inputs):
    """Wall-clock the NEFF execution with device-resident inputs (ns)."""
    import jax
    import jax.numpy as jnp
    from jax.experimental.shard_map import shard_map
    from jax.sharding import Mesh, PartitionSpec
    import concourse.bass2jax as bass2jax
    from concourse.bass2jax import _bass_exec_p, install_neuronx_cc_hook

    x = np.asarray(inputs["x"], dtype=np.float32)
    prep = _preprocess(x, np.asarray(inputs["edge_index"]))
    wcat, wofs = _pack_weights(inputs)
    pcat, pofs = _pack_params(inputs)
    nc = _get_program(prep, wcat.shape[1], pcat.shape[1], wofs, pofs)
    in_maps = []
    for c in range(NCORES):
        in_maps.append({
            "xT": prep["xT"][c], "idx_all": prep["idx_all"][c],
            "seg_all": prep["seg_all"][c], "icnt": prep["icnt"][c],
            "wcat": wcat, "pcat": pcat,
        })

    install_neuronx_cc_hook()
    from concourse.bass2jax import partition_id_tensor
    partition_name = nc.partition_id_tensor.name if nc.partition_id_tensor else None
    in_names, out_names, out_avals, zero_outs = [], [], [], []
    for alloc in nc.m.functions[0].allocations:
        if not isinstance(alloc, mybir.MemoryLocationSet):
            continue
        name = alloc.memorylocations[0].name
        if alloc.kind == "ExternalInput":
            if name != partition_name:
                in_names.append(name)
        elif alloc.kind == "ExternalOutput":
            shape = tuple(alloc.tensor_shape)
            dtype = mybir.dt.np(alloc.dtype)
            out_names.append(name)
            out_avals.append(jax.core.ShapedArray(shape, dtype))
            zero_outs.append(np.zeros(shape, dtype))
    n_params = len(in_names)
    n_outs = len(out_avals)
    in_names = in_names + out_names

    def _body(*args):
        operands = list(args)
        if partition_name is not None:
            operands.append(partition_id_tensor())
        outs = _bass_exec_p.bind(
            *operands, out_avals=tuple(out_avals), in_names=tuple(in_names + ([partition_name] if partition_name else [])),
            out_names=tuple(out_names), lowering_input_output_aliases=(),
            sim_require_finite=True, sim_require_nnan=True, nc=nc)
        return tuple(outs)

    devices = jax.devices()[:NCORES]
    mesh = Mesh(np.asarray(devices), ("core",))
    in_specs = (PartitionSpec("core"),) * (n_params + n_outs)
    out_specs = (PartitionSpec("core"),) * n_outs
    donate = tuple(range(n_params, n_params + n_outs))
    sharded = jax.jit(shard_map(_body, mesh=mesh, in_specs=in_specs,
                                out_specs=out_specs, check_rep=False),
                      donate_argnums=donate, keep_unused=True)
    concat_in = [np.concatenate([np.asarray(in_maps[c][in_names[i]])
                                 for c in range(NCORES)], axis=0)
                 for i in range(n_params)]
    sharding = jax.sharding.NamedSharding(mesh, PartitionSpec("core"))
    dev_in = [jax.device_put(a, sharding) for a in concat_in]

    def make_zeros():
        zs = [jax.device_put(np.zeros((NCORES * z.shape[0], *z.shape[1:]), z.dtype),
                             sharding) for z in zero_outs]
        jax.block_until_ready(zs)
        return zs

    # warm-up (compiles)
    out = sharded(*dev_in, *make_zeros())
    jax.block_until_ready(out)

    # Pipelined measurement: enqueue k executions back-to-back and block
    # once. The slope d(total)/dk is the per-execution device time with the
    # host/tunnel dispatch latency amortized out (dispatch overlaps the
    # device queue). Report the best slope over a few trials.
    def run_batch(k):
        zsets = [make_zeros() for _ in range(k)]
        t0 = time.perf_counter()
        outs = []
        for zs in zsets:
            outs.append(sharded(*dev_in, *zs))
        jax.block_until_ready(outs)
        t1 = time.perf_counter()
        return t1 - t0

    k_lo, k_hi = 1, 25
    slopes = []
    for _ in range(max(reps, 5)):
        t_lo = run_batch(k_lo)
        t_hi = run_batch(k_hi)
        slopes.append((t_hi - t_lo) / (k_hi - k_lo))
    slopes.sort()
    med = slopes[len(slopes) // 2]
    return med * 1e9


import time  # noqa: E402


if __name__ == "__main__":
    pass


